# revision 3
# baseline (speedup 1.0000x reference)
"""Trainium2 Bass kernel for BoTNet-style attention (nn_Attention_87436944212609).

Reference computation (per batch b of 4, heads=4, dim_head=64, fmap 64x64):
  qkv = 1x1 conv of fmap (256ch) with w_qkv (768,256)
  q,k,v = split; raw-reshaped to (heads, hw=4096, 64)  [BoTNet .view quirk:
          sequence index i = (channel_in_head, y), feature dim d = x]
  sim = q*scale @ k^T + q*scale @ emb^T   (emb[j] = pos_height[j//64]+pos_width[j%64])
  out = softmax(sim) @ v, raw-reshaped back to (256, 64, 64)

Sharding: 8 cores = 4 batches x 2 head-pairs. Each core computes its batch's
qkv projection for its 2 heads and runs full 4096x4096 attention for each head.

Algebraic folds:
  - scale folded into W_q rows (host-side slice prep)
  - emb folded into K:  q·k + q·emb = q·(k+emb); embT built on-device from
    pos_height^T/pos_width^T with broadcast APs
  - softmax denominator via ones-column appended to V (row 64 of O^T)
  - no row-max subtraction (scores ~ N(0,1); exp range safe in fp32/bf16)
"""

import sys

sys.path.insert(0, "/opt/trn_rl_repo")

from contextlib import ExitStack

import numpy as np

import concourse.bass as bass
import concourse.tile as tile
from concourse import bacc, mybir
from concourse.bass import ts
from concourse.bass_utils import run_bass_kernel_spmd
from concourse.masks import make_identity

F32 = mybir.dt.float32
BF16 = mybir.dt.bfloat16

N_CORES = 8
SCALE = 64 ** -0.5  # dim_head ** -0.5


def _emit_body(tc, pools, aps, rep):
    """One full iteration of the kernel (conv + 2 heads of attention)."""
    nc = tc.nc
    singles, temps, pP, pOsb, pout, psmall, pS, pO = pools
    fm, wt, pht, pwt, out, id16, idf = aps

    # ---- load + cast inputs ----
    fm_f = temps.tile([128, 2, 4096], F32, tag="fm_f")
    nc.sync.dma_start(fm_f[:], fm.ap().rearrange("(t p) s -> p t s", p=128))
    F16 = temps.tile([128, 2, 4096], BF16, tag="F16")
    nc.vector.tensor_copy(F16[:], fm_f[:])

    wt_f = temps.tile([128, 2, 384], F32, tag="wt_f")
    nc.sync.dma_start(wt_f[:], wt.ap().rearrange("(t p) n -> p t n", p=128))
    WT16 = temps.tile([128, 2, 384], BF16, tag="WT16")
    nc.vector.tensor_copy(WT16[:], wt_f[:])

    pht_f = temps.tile([64, 64], F32, tag="pht_f")
    nc.sync.dma_start(pht_f[:], pht.ap())
    pwt_f = temps.tile([64, 64], F32, tag="pwt_f")
    nc.sync.dma_start(pwt_f[:], pwt.ap())

    # embT[x, o*64+y] = pht[x,o] + pwt[x,y]  (bf16, 64 x 4096)
    embT = temps.tile([64, 64, 64], BF16, tag="embT")
    in0 = bass.AP(pht_f.tensor, pht_f.offset, [pht_f.ap[0], pht_f.ap[1], [0, 64]])
    in1 = bass.AP(pwt_f.tensor, pwt_f.offset, [pwt_f.ap[0], [0, 64], pwt_f.ap[1]])
    nc.vector.tensor_add(embT[:], in0, in1)

    # ---- qkv conv, transposed output layout ----
    # QKVT[x, seg, o*64+y] = sum_c fm[c, y*64+x] * WT[c, seg*64+o]
    # segs: 0=qA 1=qB 2=kA 3=kB 4=vA 5=vB  (q pre-scaled on host)
    QKVT = temps.tile([64, 6, 64, 64], BF16, tag="QKVT")
    for y in range(64):
        psc = psmall.tile([64, 384], F32, tag="ps_small")
        nc.tensor.matmul(
            psc[:], lhsT=F16[:, 0, ts(y, 64)], rhs=WT16[:, 0, :],
            start=True, stop=False,
        )
        nc.tensor.matmul(
            psc[:], lhsT=F16[:, 1, ts(y, 64)], rhs=WT16[:, 1, :],
            start=False, stop=True,
        )
        nc.vector.tensor_copy(
            QKVT[:, :, :, y], psc.rearrange("p (s o) -> p s o", s=6)
        )

    # K' = K + emb (per head)
    KTp = temps.tile([64, 2, 64, 64], BF16, tag="KTp")
    for hh in range(2):
        nc.vector.tensor_add(KTp[:, hh], QKVT[:, 2 + hh], embT[:])

    # ---- V tiles: VA[(o2,y), jt, x], with ones column at x=64 ----
    VA = temps.tile([128, 2, 32, 65], BF16, tag="VA")
    nc.vector.memset(VA[:, :, :, 64], 1.0)
    for hh in range(2):
        vt = QKVT[:, 4 + hh].rearrange("p a b -> p (a b)")
        for jt in range(32):
            pvt = psmall.tile([128, 64], BF16, tag="ps_small")
            nc.tensor.transpose(pvt[:], vt[:, ts(jt, 128)], id16[0:64, 0:64])
            nc.vector.tensor_copy(VA[:, hh, jt, 0:64], pvt[:])

    # ---- attention per head ----
    for hh in range(2):
        qt = QKVT[:, hh].rearrange("p a b -> p (a b)")
        kt = KTp[:, hh].rearrange("p a b -> p (a b)")
        for ic in range(4):  # i chunks of 1024
            psO = pO.tile([128, 1024], F32, tag="psO")
            for jb in range(32):  # j blocks of 128
                psS = pS.tile([128, 1024], F32, tag="psS")
                P16 = pP.tile([128, 1024], BF16, tag="P16")
                for n in range(2):
                    nc.tensor.matmul(
                        psS[:, ts(n, 512)],
                        lhsT=kt[:, ts(jb, 128)],
                        rhs=qt[:, ic * 1024 + n * 512 : ic * 1024 + (n + 1) * 512],
                        start=True, stop=True,
                    )
                nc.scalar.activation(
                    P16[:], psS[:], mybir.ActivationFunctionType.Exp
                )
                for n in range(2):
                    nc.tensor.matmul(
                        psO[0:65, ts(n, 512)],
                        lhsT=VA[:, hh, jb, :],
                        rhs=P16[:, ts(n, 512)],
                        start=(jb == 0), stop=(jb == 31),
                        skip_group_check=True,
                    )
            # ---- finalize chunk: transpose to i-major, divide by rowsum ----
            o_sb = pOsb.tile([65, 1024], F32, tag="o_sb")
            nc.vector.tensor_copy(o_sb[:], psO[0:65, :])
            for t in range(8):
                T = ic * 8 + t  # global i-tile: rows o in {2T, 2T+1}
                psT = psmall.tile([128, 65], F32, tag="ps_small")
                nc.tensor.transpose(psT[:], o_sb[:, ts(t, 128)], idf[0:65, 0:65])
                rr = pout.tile([128, 1], F32, tag="rr")
                nc.vector.reciprocal(rr[:], psT[:, 64:65])
                ot = pout.tile([128, 64], F32, tag="ot")
                nc.vector.tensor_scalar_mul(ot[:], psT[:, 0:64], rr[:])
                dst = out.ap()[hh * 64 + 2 * T : hh * 64 + 2 * T + 2, :].rearrange(
                    "c (y x) -> (c y) x", x=64
                )
                nc.sync.dma_start(dst, ot[:])


def build_module(reps=1):
    nc = bacc.Bacc("TRN2", target_bir_lowering=False, debug=False)
    fm = nc.dram_tensor("fm", [256, 4096], F32, kind="ExternalInput")
    wt = nc.dram_tensor("wt", [256, 384], F32, kind="ExternalInput")
    pht = nc.dram_tensor("pht", [64, 64], F32, kind="ExternalInput")
    pwt = nc.dram_tensor("pwt", [64, 64], F32, kind="ExternalInput")
    out = nc.dram_tensor("out", [128, 4096], F32, kind="ExternalOutput")

    with tile.TileContext(nc) as tc:
        with ExitStack() as ctx:
            singles = ctx.enter_context(tc.tile_pool(name="singles", bufs=1))
            temps = ctx.enter_context(tc.tile_pool(name="temps", bufs=1))
            pP = ctx.enter_context(tc.tile_pool(name="pP", bufs=3))
            pOsb = ctx.enter_context(tc.tile_pool(name="pOsb", bufs=2))
            pout = ctx.enter_context(tc.tile_pool(name="pout", bufs=3))
            psmall = ctx.enter_context(tc.tile_pool(name="psmall", bufs=2, space="PSUM"))
            pS = ctx.enter_context(tc.tile_pool(name="pS", bufs=2, space="PSUM"))
            pO = ctx.enter_context(tc.tile_pool(name="pO", bufs=1, space="PSUM"))

            id16 = singles.tile([128, 128], BF16)
            make_identity(nc, id16)
            idf = singles.tile([128, 128], F32)
            make_identity(nc, idf)

            pools = (singles, temps, pP, pOsb, pout, psmall, pS, pO)
            aps = (fm, wt, pht, pwt, out, id16, idf)
            for rep in range(reps):
                _emit_body(tc, pools, aps, rep)
    nc.compile()
    return nc


def build_module_loop(loop_reps):
    """Same kernel, body wrapped in an on-device For_i loop (for timing)."""
    nc = bacc.Bacc("TRN2", target_bir_lowering=False, debug=False)
    fm = nc.dram_tensor("fm", [256, 4096], F32, kind="ExternalInput")
    wt = nc.dram_tensor("wt", [256, 384], F32, kind="ExternalInput")
    pht = nc.dram_tensor("pht", [64, 64], F32, kind="ExternalInput")
    pwt = nc.dram_tensor("pwt", [64, 64], F32, kind="ExternalInput")
    out = nc.dram_tensor("out", [128, 4096], F32, kind="ExternalOutput")

    with tile.TileContext(nc) as tc:
        with ExitStack() as ctx:
            singles = ctx.enter_context(tc.tile_pool(name="singles", bufs=1))
            temps = ctx.enter_context(tc.tile_pool(name="temps", bufs=1))
            pP = ctx.enter_context(tc.tile_pool(name="pP", bufs=3))
            pOsb = ctx.enter_context(tc.tile_pool(name="pOsb", bufs=2))
            pout = ctx.enter_context(tc.tile_pool(name="pout", bufs=3))
            psmall = ctx.enter_context(tc.tile_pool(name="psmall", bufs=2, space="PSUM"))
            pS = ctx.enter_context(tc.tile_pool(name="pS", bufs=2, space="PSUM"))
            pO = ctx.enter_context(tc.tile_pool(name="pO", bufs=1, space="PSUM"))

            id16 = singles.tile([128, 128], BF16)
            make_identity(nc, id16)
            idf = singles.tile([128, 128], F32)
            make_identity(nc, idf)

            pools = (singles, temps, pP, pOsb, pout, psmall, pS, pO)
            aps = (fm, wt, pht, pwt, out, id16, idf)
            with tc.For_i(0, loop_reps, 1):
                _emit_body(tc, pools, aps, 0)
    nc.compile()
    return nc


_module_cache = {}


def get_module(reps=1):
    if reps not in _module_cache:
        _module_cache[reps] = build_module(reps)
    return _module_cache[reps]


def get_module_loop(loop_reps):
    key = ("loop", loop_reps)
    if key not in _module_cache:
        _module_cache[key] = build_module_loop(loop_reps)
    return _module_cache[key]


def make_in_maps(fmap, w_qkv, pos_height, pos_width):
    in_maps = []
    for c in range(N_CORES):
        b, p = c // 2, c % 2
        hA = 2 * p
        segs = []
        for s in range(3):  # q, k, v
            for h in (hA, hA + 1):
                rows = np.asarray(w_qkv[s * 256 + h * 64 : s * 256 + h * 64 + 64, :])
                if s == 0:
                    rows = rows * SCALE
                segs.append(rows)
        wt = np.ascontiguousarray(np.concatenate(segs, 0).T, dtype=np.float32)
        in_maps.append(
            {
                "fm": np.ascontiguousarray(
                    np.asarray(fmap[b]).reshape(256, 4096), dtype=np.float32
                ),
                "wt": wt,
                "pht": np.ascontiguousarray(np.asarray(pos_height).T, dtype=np.float32),
                "pwt": np.ascontiguousarray(np.asarray(pos_width).T, dtype=np.float32),
            }
        )
    return in_maps


def kernel(fmap, w_qkv, pos_height, pos_width):
    nc = get_module(reps=1)
    in_maps = make_in_maps(fmap, w_qkv, pos_height, pos_width)
    res = run_bass_kernel_spmd(nc, in_maps, core_ids=list(range(N_CORES)))
    out = np.empty((4, 256, 64, 64), np.float32)
    for c in range(N_CORES):
        b, p = c // 2, c % 2
        out[b, 128 * p : 128 * p + 128] = res.results[c]["out"].reshape(128, 64, 64)
    return out


# revision 15
# speedup vs baseline: 3.9785x; 3.9785x over previous
"""Trainium2 Bass kernel for BoTNet-style attention (nn_Attention_87436944212609).

Reference computation (per batch b of 4, heads=4, dim_head=64, fmap 64x64):
  qkv = 1x1 conv of fmap (256ch) with w_qkv (768,256)
  q,k,v = split; raw-reshaped to (heads, hw=4096, 64)  [BoTNet .view quirk:
          sequence index i = (channel_in_head, y), feature dim d = x]
  sim = q*scale @ k^T + q*scale @ emb^T   (emb[j] = pos_height[j//64]+pos_width[j%64])
  out = softmax(sim) @ v, raw-reshaped back to (256, 64, 64)

Sharding: 8 cores = 4 batches x 2 head-pairs. Each core computes its batch's
qkv projection for its 2 heads and runs full 4096x4096 attention for each head.

Algebraic folds:
  - scale folded into W_q rows (host-side slice prep)
  - emb folded into K:  q·k + q·emb = q·(k+emb); embT built on-device from
    pos_height^T/pos_width^T with broadcast APs, added during the conv copy
  - softmax denominator via ones-column appended to V (row 64 of O^T)
  - no row-max subtraction (scores ~ N(0,1); exp range safe in fp32)

All matmuls run in float32r (TF32-like, full PE rate at moving dim >= 256).
"""

import sys

sys.path.insert(0, "/opt/trn_rl_repo")

from contextlib import ExitStack

import numpy as np

import concourse.bass as bass
import concourse.tile as tile
from concourse import bacc, mybir
from concourse.bass import ts
from concourse.bass_utils import run_bass_kernel_spmd
from concourse.masks import make_identity

F32 = mybir.dt.float32
F32R = mybir.dt.float32r
BF16 = mybir.dt.bfloat16

N_CORES = 8
SCALE = 64 ** -0.5  # dim_head ** -0.5


def _emit_body(tc, pools, aps, rep):
    """One full iteration of the kernel (conv + 2 heads of attention)."""
    nc = tc.nc
    singles, temps, pP, pOsb, pout, psmall, pS, pO = pools
    fm, wt, pht, pwt, out, idr, idf = aps

    # ---- load inputs (all f32r so the PE runs full-rate fp32) ----
    fm_f = temps.tile([128, 2, 4096], F32R, tag="fm_f")
    nc.sync.dma_start(fm_f[:], fm.ap().rearrange("(t p) s -> p t s", p=128))
    wt_f = temps.tile([128, 2, 384], F32R, tag="wt_f")
    nc.sync.dma_start(wt_f[:], wt.ap().rearrange("(t p) n -> p t n", p=128))
    pht_f = temps.tile([64, 64], F32R, tag="pht_f")
    nc.sync.dma_start(pht_f[:], pht.ap())
    pwt_f = temps.tile([64, 64], F32R, tag="pwt_f")
    nc.sync.dma_start(pwt_f[:], pwt.ap())

    # embT[x, o*64+y] = pht[x,o] + pwt[x,y]  (64 x 4096)
    embT = temps.tile([64, 64, 64], F32R, tag="embT")
    in0 = bass.AP(pht_f.tensor, pht_f.offset, [pht_f.ap[0], pht_f.ap[1], [0, 64]])
    in1 = bass.AP(pwt_f.tensor, pwt_f.offset, [pwt_f.ap[0], [0, 64], pwt_f.ap[1]])
    nc.vector.tensor_add(embT[:], in0, in1)

    # ---- qkv conv, transposed output layout; emb added to k on the fly ----
    # QKVT[x, seg, o*64+y] = sum_c fm[c, y*64+x] * WT[c, seg*64+o]  (+ embT on k)
    # segs: 0=qA 1=qB 2=k'A 3=k'B 4=vA 5=vB  (q pre-scaled on host)
    QKVT = temps.tile([64, 6, 64, 64], F32R, tag="QKVT")
    for y in range(64):
        psc = psmall.tile([64, 384], F32, tag="ps_small")
        nc.tensor.matmul(
            psc[:], lhsT=fm_f[:, 0, ts(y, 64)], rhs=wt_f[:, 0, :],
            start=True, stop=False,
        )
        nc.tensor.matmul(
            psc[:], lhsT=fm_f[:, 1, ts(y, 64)], rhs=wt_f[:, 1, :],
            start=False, stop=True,
        )
        pv = psc.rearrange("p (s o) -> p s o", s=6)
        nc.vector.tensor_copy(QKVT[:, 0:2, :, y], pv[:, 0:2])
        emb_b = bass.AP(
            embT.tensor, embT.offset + embT.ap[2][0] * y,
            [embT.ap[0], [0, 2], embT.ap[1]],
        )
        nc.vector.tensor_add(QKVT[:, 2:4, :, y], pv[:, 2:4], emb_b)
        nc.vector.tensor_copy(QKVT[:, 4:6, :, y], pv[:, 4:6])

    # ---- V tiles: VA[(o2,y), jt, x], with ones column at x=64 ----
    VA = temps.tile([128, 2, 32, 65], F32R, tag="VA")
    nc.vector.memset(VA[:, :, :, 64].bitcast(F32), 1.0)
    for hh in range(2):
        vt = QKVT[:, 4 + hh].rearrange("p a b -> p (a b)")
        for jt in range(32):
            pvt = psmall.tile([128, 64], F32R, tag="ps_small")
            nc.tensor.transpose(
                pvt[:], vt[:, ts(jt, 128)], idr[0:64, 0:64]
            )
            nc.vector.tensor_copy(VA[:, hh, jt, 0:64], pvt[:])

    # ---- attention per head ----
    # Two 1024-wide i-chunks per pass so 4 consecutive S matmuls share kt[jb]
    # and 4 consecutive O matmuls share VA[jb] (weight reload amortization).
    for hh in range(2):
        qt = QKVT[:, hh].rearrange("p a b -> p (a b)")
        kt = QKVT[:, 2 + hh].rearrange("p a b -> p (a b)")
        for ic in range(4):  # i chunks of 1024
            psO = pO.tile([128, 1024], F32, tag="psO")
            for jb in range(32):  # j blocks of 128
                psS = pS.tile([128, 1024], F32, tag="psS")
                P32 = pP.tile([128, 1024], F32R, tag="P32")
                for n in range(2):
                    nc.tensor.matmul(
                        psS[:, ts(n, 512)],
                        lhsT=kt[:, ts(jb, 128)],
                        rhs=qt[:, ic * 1024 + n * 512 : ic * 1024 + (n + 1) * 512],
                        start=True, stop=True,
                    )
                nc.scalar.activation(
                    P32[:], psS[:], mybir.ActivationFunctionType.Exp
                )
                for n in range(2):
                    nc.tensor.matmul(
                        psO[0:65, ts(n, 512)],
                        lhsT=VA[:, hh, jb, :],
                        rhs=P32[:, ts(n, 512)],
                        start=(jb == 0), stop=(jb == 31),
                        skip_group_check=True,
                    )
            # ---- finalize chunk: transpose to i-major, divide by rowsum ----
            if True:
                o_sb = pOsb.tile([65, 1024], F32, tag="o_sb")
                nc.vector.tensor_copy(o_sb[:], psO[0:65, :])
                for t in range(8):
                    T = ic * 8 + t  # global i-tile: rows o in {2T, 2T+1}
                    psT = psmall.tile([128, 65], F32, tag="ps_small")
                    nc.tensor.transpose(psT[:], o_sb[:, ts(t, 128)], idf[0:65, 0:65])
                    rr = pout.tile([128, 1], F32, tag="rr")
                    nc.vector.reciprocal(rr[:], psT[:, 64:65])
                    ot = pout.tile([128, 64], F32, tag="ot")
                    nc.vector.tensor_scalar_mul(ot[:], psT[:, 0:64], rr[:])
                    dst = out.ap()[hh * 64 + 2 * T : hh * 64 + 2 * T + 2, :].rearrange(
                        "c (y x) -> (c y) x", x=64
                    )
                    nc.sync.dma_start(dst, ot[:])


def _emit_body_rt(tc, pools, aps, rep):
    """Row-tiled variant: q/k replicated on both partition halves; S matmuls
    for two j-blocks run concurrently in opposite PE row groups."""
    nc = tc.nc
    singles, temps, pP, pOsb, pout, psmall, pS, pO = pools
    fm, wt, pht, pwt, out, idr, idf = aps

    fm_f = temps.tile([128, 2, 4096], F32R, tag="fm_f")
    nc.sync.dma_start(fm_f[:], fm.ap().rearrange("(t p) s -> p t s", p=128))
    wt_f = temps.tile([128, 2, 384], F32R, tag="wt_f")
    nc.sync.dma_start(wt_f[:], wt.ap().rearrange("(t p) n -> p t n", p=128))
    # pos embeddings, replicated onto both partition halves
    pht_f = temps.tile([64, 64], F32R, tag="pht_f")
    nc.sync.dma_start(pht_f[:], pht.ap())
    pwt_f = temps.tile([64, 64], F32R, tag="pwt_f")
    nc.sync.dma_start(pwt_f[:], pwt.ap())

    embT = temps.tile([64, 64, 64], F32R, tag="embT")
    in0 = bass.AP(pht_f.tensor, pht_f.offset, [pht_f.ap[0], pht_f.ap[1], [0, 64]])
    in1 = bass.AP(pwt_f.tensor, pwt_f.offset, [pwt_f.ap[0], [0, 64], pwt_f.ap[1]])
    nc.vector.tensor_add(embT[:], in0, in1)

    # conv with duplicated output on both partition halves (col-tiled pairs)
    QKVT = temps.tile([128, 6, 64, 64], F32R, tag="QKVT")
    for y in range(64):
        psc = psmall.tile([64, 384], F32, tag="ps_small")
        nc.tensor.matmul(
            psc[:], lhsT=fm_f[:, 0, ts(y, 64)], rhs=wt_f[:, 0, :],
            start=True, stop=False,
        )
        nc.tensor.matmul(
            psc[:], lhsT=fm_f[:, 1, ts(y, 64)], rhs=wt_f[:, 1, :],
            start=False, stop=True,
        )
        pv = psc.rearrange("p (s o) -> p s o", s=6)
        nc.vector.tensor_copy(QKVT[0:64, 0:2, :, y], pv[:, 0:2])
        emb_b = bass.AP(
            embT.tensor, embT.offset + embT.ap[2][0] * y,
            [[embT.ap[0][0], 64], [0, 2], embT.ap[1]],
        )
        nc.vector.tensor_add(QKVT[0:64, 2:4, :, y], pv[:, 2:4], emb_b)
        nc.vector.tensor_copy(QKVT[0:64, 4:6, :, y], pv[:, 4:6])
    # replicate q and k' onto the upper partition half for row-tiled matmuls
    nc.sync.dma_start(QKVT[64:128, 0:4], QKVT[0:64, 0:4])

    VA = temps.tile([128, 2, 32, 65], F32R, tag="VA")
    nc.vector.memset(VA[:, :, :, 64].bitcast(F32), 1.0)
    for hh in range(2):
        vt = QKVT[0:64, 4 + hh].rearrange("p a b -> p (a b)")
        for jt in range(32):
            pvt = psmall.tile([128, 64], F32R, tag="ps_small")
            nc.tensor.transpose(pvt[:], vt[:, ts(jt, 128)], idr[0:64, 0:64])
            nc.vector.tensor_copy(VA[:, hh, jt, 0:64], pvt[:])

    for hh in range(2):
        qt = QKVT[:, hh].rearrange("p a b -> p (a b)")
        kt = QKVT[:, 2 + hh].rearrange("p a b -> p (a b)")
        for ic in range(8):  # i chunks of 512
            psO = pO.tile([128, 512], F32, tag="psO")
            for jp in range(16):  # pairs of j blocks
                pSt = [pS.tile([128, 512], F32, tag="psS", name=f"psS_{h2}")
                       for h2 in range(2)]
                pPt = [pP.tile([128, 512], F32R, tag="P32", name=f"P32_{h2}")
                       for h2 in range(2)]
                for h2 in range(2):
                    jb = 2 * jp + h2
                    lo, hi = 64 * h2, 64 * h2 + 64
                    nc.tensor.matmul(
                        pSt[h2][:],
                        lhsT=kt[lo:hi, ts(jb, 128)],
                        rhs=qt[lo:hi, ts(ic, 512)],
                        start=True, stop=True,
                    )
                for h2 in range(2):
                    nc.scalar.activation(
                        pPt[h2][:], pSt[h2][:], mybir.ActivationFunctionType.Exp
                    )
                for h2 in range(2):
                    jb = 2 * jp + h2
                    nc.tensor.matmul(
                        psO[0:65, :],
                        lhsT=VA[:, hh, jb, :],
                        rhs=pPt[h2][:],
                        start=(jb == 0), stop=(jb == 31),
                        skip_group_check=True,
                    )
            o_sb = pOsb.tile([65, 512], F32, tag="o_sb")
            nc.vector.tensor_copy(o_sb[:], psO[0:65, :])
            for t in range(4):
                T = ic * 4 + t
                psT = psmall.tile([128, 65], F32, tag="ps_small")
                nc.tensor.transpose(psT[:], o_sb[:, ts(t, 128)], idf[0:65, 0:65])
                rr = pout.tile([128, 1], F32, tag="rr")
                nc.vector.reciprocal(rr[:], psT[:, 64:65])
                ot = pout.tile([128, 64], F32, tag="ot")
                nc.vector.tensor_scalar_mul(ot[:], psT[:, 0:64], rr[:])
                dst = out.ap()[hh * 64 + 2 * T : hh * 64 + 2 * T + 2, :].rearrange(
                    "c (y x) -> (c y) x", x=64
                )
                nc.sync.dma_start(dst, ot[:])


def _build(loop_reps, static_reps=1, rt=False):
    nc = bacc.Bacc("TRN2", target_bir_lowering=False, debug=False)
    fm = nc.dram_tensor("fm", [256, 4096], F32R, kind="ExternalInput")
    wt = nc.dram_tensor("wt", [256, 384], F32R, kind="ExternalInput")
    pht = nc.dram_tensor("pht", [64, 64], F32R, kind="ExternalInput")
    pwt = nc.dram_tensor("pwt", [64, 64], F32R, kind="ExternalInput")
    out = nc.dram_tensor("out", [128, 4096], F32, kind="ExternalOutput")

    with tile.TileContext(nc) as tc:
        with ExitStack() as ctx:
            singles = ctx.enter_context(tc.tile_pool(name="singles", bufs=1))
            temps = ctx.enter_context(tc.tile_pool(name="temps", bufs=1))
            pP = ctx.enter_context(tc.tile_pool(name="pP", bufs=4))
            pOsb = ctx.enter_context(tc.tile_pool(name="pOsb", bufs=2))
            pout = ctx.enter_context(tc.tile_pool(name="pout", bufs=3))
            psmall = ctx.enter_context(tc.tile_pool(name="psmall", bufs=2, space="PSUM"))
            pS = ctx.enter_context(tc.tile_pool(name="pS", bufs=4 if rt else 2, space="PSUM"))
            pO = ctx.enter_context(tc.tile_pool(name="pO", bufs=2 if rt else 1, space="PSUM"))

            idf = singles.tile([128, 128], F32)
            make_identity(nc, idf)
            idr = singles.tile([128, 128], F32R)
            nc.vector.tensor_copy(idr[:], idf[:])

            pools = (singles, temps, pP, pOsb, pout, psmall, pS, pO)
            aps = (fm, wt, pht, pwt, out, idr, idf)
            body = _emit_body_rt if rt else _emit_body
            if loop_reps > 1:
                with tc.For_i(0, loop_reps, 1):
                    body(tc, pools, aps, 0)
            else:
                for rep in range(static_reps):
                    body(tc, pools, aps, rep)
    nc.compile()
    return nc


def build_module(reps=1):
    return _build(1, static_reps=reps)


def build_module_loop(loop_reps):
    return _build(loop_reps)


_module_cache = {}


def get_module(reps=1):
    if reps not in _module_cache:
        _module_cache[reps] = build_module(reps)
    return _module_cache[reps]


def get_module_loop(loop_reps):
    key = ("loop", loop_reps)
    if key not in _module_cache:
        _module_cache[key] = build_module_loop(loop_reps)
    return _module_cache[key]


def make_in_maps(fmap, w_qkv, pos_height, pos_width):
    in_maps = []
    for c in range(N_CORES):
        b, p = c // 2, c % 2
        hA = 2 * p
        segs = []
        for s in range(3):  # q, k, v
            for h in (hA, hA + 1):
                rows = np.asarray(w_qkv[s * 256 + h * 64 : s * 256 + h * 64 + 64, :])
                if s == 0:
                    rows = rows * SCALE
                segs.append(rows)
        wt = np.ascontiguousarray(np.concatenate(segs, 0).T, dtype=np.float32)
        in_maps.append(
            {
                "fm": np.ascontiguousarray(
                    np.asarray(fmap[b]).reshape(256, 4096), dtype=np.float32
                ),
                "wt": wt,
                "pht": np.ascontiguousarray(np.asarray(pos_height).T, dtype=np.float32),
                "pwt": np.ascontiguousarray(np.asarray(pos_width).T, dtype=np.float32),
            }
        )
    return in_maps


def kernel(fmap, w_qkv, pos_height, pos_width):
    nc = get_module(reps=1)
    in_maps = make_in_maps(fmap, w_qkv, pos_height, pos_width)
    res = run_bass_kernel_spmd(nc, in_maps, core_ids=list(range(N_CORES)))
    out = np.empty((4, 256, 64, 64), np.float32)
    for c in range(N_CORES):
        b, p = c // 2, c % 2
        out[b, 128 * p : 128 * p + 128] = res.results[c]["out"].reshape(128, 64, 64)
    return out


# revision 17
# speedup vs baseline: 4.2696x; 1.0732x over previous
"""Trainium2 Bass kernel for BoTNet-style attention (nn_Attention_87436944212609).

Reference computation (per batch b of 4, heads=4, dim_head=64, fmap 64x64):
  qkv = 1x1 conv of fmap (256ch) with w_qkv (768,256)
  q,k,v = split; raw-reshaped to (heads, hw=4096, 64)  [BoTNet .view quirk:
          sequence index i = (channel_in_head, y), feature dim d = x]
  sim = q*scale @ k^T + q*scale @ emb^T   (emb[j] = pos_height[j//64]+pos_width[j%64])
  out = softmax(sim) @ v, raw-reshaped back to (256, 64, 64)

Sharding: 8 cores = 4 batches x 2 head-pairs. Each core computes its batch's
qkv projection for its 2 heads and runs full 4096x4096 attention for each head.

Algebraic folds:
  - scale folded into W_q rows (host-side slice prep)
  - emb folded into K:  q·k + q·emb = q·(k+emb); embT built on-device from
    pos_height^T/pos_width^T with broadcast APs, added during the conv copy
  - softmax denominator via ones-column appended to V (row 64 of O^T)
  - no row-max subtraction (scores ~ N(0,1); exp range safe in fp32)

All matmuls run in float32r (TF32-like, full PE rate at moving dim >= 256).
"""

import sys

sys.path.insert(0, "/opt/trn_rl_repo")

from contextlib import ExitStack

import numpy as np

import concourse.bass as bass
import concourse.tile as tile
from concourse import bacc, mybir
from concourse.bass import ts
from concourse.bass_utils import run_bass_kernel_spmd
from concourse.masks import make_identity

F32 = mybir.dt.float32
F32R = mybir.dt.float32r
BF16 = mybir.dt.bfloat16

N_CORES = 8
SCALE = 64 ** -0.5  # dim_head ** -0.5
ATTN_V2 = False


def _emit_body(tc, pools, aps, rep):
    """One full iteration of the kernel (conv + 2 heads of attention)."""
    nc = tc.nc
    singles, temps, pP, pOsb, pout, psmall, pS, pO = pools
    fm, wt, pht, pwt, out, idr, idf = aps

    # ---- load inputs (all f32r so the PE runs full-rate fp32) ----
    fm_f = temps.tile([128, 2, 4096], F32R, tag="fm_f")
    nc.sync.dma_start(fm_f[:], fm.ap().rearrange("(t p) s -> p t s", p=128))
    wt_f = temps.tile([128, 2, 384], F32R, tag="wt_f")
    nc.sync.dma_start(wt_f[:], wt.ap().rearrange("(t p) n -> p t n", p=128))
    pht_f = temps.tile([64, 64], F32R, tag="pht_f")
    nc.sync.dma_start(pht_f[:], pht.ap())
    pwt_f = temps.tile([64, 64], F32R, tag="pwt_f")
    nc.sync.dma_start(pwt_f[:], pwt.ap())

    # embT[x, o*64+y] = pht[x,o] + pwt[x,y]  (64 x 4096)
    embT = temps.tile([64, 64, 64], F32R, tag="embT")
    in0 = bass.AP(pht_f.tensor, pht_f.offset, [pht_f.ap[0], pht_f.ap[1], [0, 64]])
    in1 = bass.AP(pwt_f.tensor, pwt_f.offset, [pwt_f.ap[0], [0, 64], pwt_f.ap[1]])
    nc.vector.tensor_add(embT[:], in0, in1)

    # ---- qkv conv, transposed output layout; emb added to k on the fly ----
    # QKVT[x, seg, o*64+y] = sum_c fm[c, y*64+x] * WT[c, seg*64+o]  (+ embT on k)
    # segs: 0=qA 1=qB 2=k'A 3=k'B 4=vA 5=vB  (q pre-scaled on host)
    QKVT = temps.tile([64, 6, 64, 64], F32R, tag="QKVT")
    for y in range(64):
        psc = psmall.tile([64, 384], F32, tag="ps_small")
        nc.tensor.matmul(
            psc[:], lhsT=fm_f[:, 0, ts(y, 64)], rhs=wt_f[:, 0, :],
            start=True, stop=False,
        )
        nc.tensor.matmul(
            psc[:], lhsT=fm_f[:, 1, ts(y, 64)], rhs=wt_f[:, 1, :],
            start=False, stop=True,
        )
        pv = psc.rearrange("p (s o) -> p s o", s=6)
        nc.vector.tensor_copy(QKVT[:, 0:2, :, y], pv[:, 0:2])
        emb_b = bass.AP(
            embT.tensor, embT.offset + embT.ap[2][0] * y,
            [embT.ap[0], [0, 2], embT.ap[1]],
        )
        nc.vector.tensor_add(QKVT[:, 2:4, :, y], pv[:, 2:4], emb_b)
        nc.vector.tensor_copy(QKVT[:, 4:6, :, y], pv[:, 4:6])

    # ---- V tiles: VA[(o2,y), jt, x], with ones column at x=64 ----
    VA = temps.tile([128, 2, 32, 65], F32R, tag="VA")
    nc.vector.memset(VA[:, :, :, 64].bitcast(F32), 1.0)
    for hh in range(2):
        vt = QKVT[:, 4 + hh].rearrange("p a b -> p (a b)")
        for jt in range(32):
            pvt = psmall.tile([128, 64], F32R, tag="ps_small")
            nc.tensor.transpose(
                pvt[:], vt[:, ts(jt, 128)], idr[0:64, 0:64]
            )
            nc.vector.tensor_copy(VA[:, hh, jt, 0:64], pvt[:])

    # ---- attention per head ----
    for hh in range(2):
        qt = QKVT[:, hh].rearrange("p a b -> p (a b)")
        kt = QKVT[:, 2 + hh].rearrange("p a b -> p (a b)")
        if ATTN_V2:
            _emit_attn_v2(tc, pools, aps, hh, qt, kt, VA, out)
            continue
        for ic in range(4):  # i chunks of 1024
            psO = pO.tile([128, 1024], F32, tag="psO")
            for jb in range(32):  # j blocks of 128
                psS = pS.tile([128, 1024], F32, tag="psS")
                P32 = pP.tile([128, 1024], F32R, tag="P32")
                for n in range(2):
                    nc.tensor.matmul(
                        psS[:, ts(n, 512)],
                        lhsT=kt[:, ts(jb, 128)],
                        rhs=qt[:, ic * 1024 + n * 512 : ic * 1024 + (n + 1) * 512],
                        start=True, stop=True,
                    )
                nc.scalar.activation(
                    P32[:], psS[:], mybir.ActivationFunctionType.Exp
                )
                for n in range(2):
                    nc.tensor.matmul(
                        psO[0:65, ts(n, 512)],
                        lhsT=VA[:, hh, jb, :],
                        rhs=P32[:, ts(n, 512)],
                        start=(jb == 0), stop=(jb == 31),
                        skip_group_check=True,
                    )
            # ---- finalize chunk: transpose to i-major, divide by rowsum ----
            if True:
                o_sb = pOsb.tile([65, 1024], F32, tag="o_sb")
                nc.vector.tensor_copy(o_sb[:], psO[0:65, :])
                for t in range(8):
                    T = ic * 8 + t  # global i-tile: rows o in {2T, 2T+1}
                    psT = psmall.tile([128, 65], F32, tag="ps_small")
                    nc.tensor.transpose(psT[:], o_sb[:, ts(t, 128)], idf[0:65, 0:65])
                    rr = pout.tile([128, 1], F32, tag="rr")
                    nc.vector.reciprocal(rr[:], psT[:, 64:65])
                    ot = pout.tile([128, 64], F32, tag="ot")
                    nc.vector.tensor_scalar_mul(ot[:], psT[:, 0:64], rr[:])
                    dst = out.ap()[hh * 64 + 2 * T : hh * 64 + 2 * T + 2, :].rearrange(
                        "c (y x) -> (c y) x", x=64
                    )
                    nc.sync.dma_start(dst, ot[:])


def _emit_attn_v2(tc, pools, aps, hh, qt, kt, VA, out):
    """Weight-amortized attention: jb outer, 4x512 i-chunks inner, so 4
    consecutive S matmuls share kt[jb] and 4 O matmuls share VA[jb].
    O accumulates in PSUM per 8-jb group, then folds into SBUF."""
    nc = tc.nc
    singles, temps, pP, pOsb, pout, psmall, pS, pO = pools
    idf = aps[-1]
    for ihalf in range(2):
        o_acc = pOsb.tile([65, 2048], F32, tag="o_acc")
        for jg in range(4):
            psOs = [pO.tile([128, 512], F32, tag="psO", name=f"psO_{i}")
                    for i in range(4)]
            for jj in range(8):
                jb = jg * 8 + jj
                psSs = [pS.tile([128, 512], F32, tag="psS", name=f"psS_{i}")
                        for i in range(2)]
                P32s = [pP.tile([128, 512], F32R, tag="P32", name=f"P32_{i}")
                        for i in range(2)]
                for half2 in range(2):
                    for ic in range(2):
                        c = half2 * 2 + ic
                        i0 = ihalf * 2048 + c * 512
                        if ic == 0:
                            psSs[0] = pS.tile([128, 512], F32, tag="psS",
                                              name=f"psS_{half2}_0")
                            psSs[1] = pS.tile([128, 512], F32, tag="psS",
                                              name=f"psS_{half2}_1")
                            P32s[0] = pP.tile([128, 512], F32R, tag="P32",
                                              name=f"P32_{half2}_0")
                            P32s[1] = pP.tile([128, 512], F32R, tag="P32",
                                              name=f"P32_{half2}_1")
                        nc.tensor.matmul(
                            psSs[ic][:], lhsT=kt[:, ts(jb, 128)],
                            rhs=qt[:, i0 : i0 + 512], start=True, stop=True,
                        )
                    for ic in range(2):
                        nc.scalar.activation(
                            P32s[ic][:], psSs[ic][:],
                            mybir.ActivationFunctionType.Exp,
                        )
                    for ic in range(2):
                        c = half2 * 2 + ic
                        nc.tensor.matmul(
                            psOs[c][0:65, :], lhsT=VA[:, hh, jb, :],
                            rhs=P32s[ic][:], start=(jj == 0), stop=(jj == 7),
                            skip_group_check=True,
                        )
            for c in range(4):
                if jg == 0:
                    nc.vector.tensor_copy(o_acc[:, ts(c, 512)], psOs[c][0:65, :])
                else:
                    nc.vector.tensor_add(
                        o_acc[:, ts(c, 512)], o_acc[:, ts(c, 512)], psOs[c][0:65, :]
                    )
        # finalize this half from o_acc
        for t in range(16):
            T = ihalf * 16 + t
            psT = psmall.tile([128, 65], F32, tag="ps_small")
            nc.tensor.transpose(psT[:], o_acc[:, ts(t, 128)], idf[0:65, 0:65])
            rr = pout.tile([128, 1], F32, tag="rr")
            nc.vector.reciprocal(rr[:], psT[:, 64:65])
            ot = pout.tile([128, 64], F32, tag="ot")
            nc.vector.tensor_scalar_mul(ot[:], psT[:, 0:64], rr[:])
            dst = out.ap()[hh * 64 + 2 * T : hh * 64 + 2 * T + 2, :].rearrange(
                "c (y x) -> (c y) x", x=64
            )
            nc.sync.dma_start(dst, ot[:])


def _emit_body_rt(tc, pools, aps, rep):
    """Row-tiled variant: q/k replicated on both partition halves; S matmuls
    for two j-blocks run concurrently in opposite PE row groups."""
    nc = tc.nc
    singles, temps, pP, pOsb, pout, psmall, pS, pO = pools
    fm, wt, pht, pwt, out, idr, idf = aps

    fm_f = temps.tile([128, 2, 4096], F32R, tag="fm_f")
    nc.sync.dma_start(fm_f[:], fm.ap().rearrange("(t p) s -> p t s", p=128))
    wt_f = temps.tile([128, 2, 384], F32R, tag="wt_f")
    nc.sync.dma_start(wt_f[:], wt.ap().rearrange("(t p) n -> p t n", p=128))
    # pos embeddings, replicated onto both partition halves
    pht_f = temps.tile([64, 64], F32R, tag="pht_f")
    nc.sync.dma_start(pht_f[:], pht.ap())
    pwt_f = temps.tile([64, 64], F32R, tag="pwt_f")
    nc.sync.dma_start(pwt_f[:], pwt.ap())

    embT = temps.tile([64, 64, 64], F32R, tag="embT")
    in0 = bass.AP(pht_f.tensor, pht_f.offset, [pht_f.ap[0], pht_f.ap[1], [0, 64]])
    in1 = bass.AP(pwt_f.tensor, pwt_f.offset, [pwt_f.ap[0], [0, 64], pwt_f.ap[1]])
    nc.vector.tensor_add(embT[:], in0, in1)

    # conv with duplicated output on both partition halves (col-tiled pairs)
    QKVT = temps.tile([128, 6, 64, 64], F32R, tag="QKVT")
    for y in range(64):
        psc = psmall.tile([64, 384], F32, tag="ps_small")
        nc.tensor.matmul(
            psc[:], lhsT=fm_f[:, 0, ts(y, 64)], rhs=wt_f[:, 0, :],
            start=True, stop=False,
        )
        nc.tensor.matmul(
            psc[:], lhsT=fm_f[:, 1, ts(y, 64)], rhs=wt_f[:, 1, :],
            start=False, stop=True,
        )
        pv = psc.rearrange("p (s o) -> p s o", s=6)
        nc.vector.tensor_copy(QKVT[0:64, 0:2, :, y], pv[:, 0:2])
        emb_b = bass.AP(
            embT.tensor, embT.offset + embT.ap[2][0] * y,
            [[embT.ap[0][0], 64], [0, 2], embT.ap[1]],
        )
        nc.vector.tensor_add(QKVT[0:64, 2:4, :, y], pv[:, 2:4], emb_b)
        nc.vector.tensor_copy(QKVT[0:64, 4:6, :, y], pv[:, 4:6])
    # replicate q and k' onto the upper partition half for row-tiled matmuls
    nc.sync.dma_start(QKVT[64:128, 0:4], QKVT[0:64, 0:4])

    VA = temps.tile([128, 2, 32, 65], F32R, tag="VA")
    nc.vector.memset(VA[:, :, :, 64].bitcast(F32), 1.0)
    for hh in range(2):
        vt = QKVT[0:64, 4 + hh].rearrange("p a b -> p (a b)")
        for jt in range(32):
            pvt = psmall.tile([128, 64], F32R, tag="ps_small")
            nc.tensor.transpose(pvt[:], vt[:, ts(jt, 128)], idr[0:64, 0:64])
            nc.vector.tensor_copy(VA[:, hh, jt, 0:64], pvt[:])

    for hh in range(2):
        qt = QKVT[:, hh].rearrange("p a b -> p (a b)")
        kt = QKVT[:, 2 + hh].rearrange("p a b -> p (a b)")
        for ic in range(8):  # i chunks of 512
            psO = pO.tile([128, 512], F32, tag="psO")
            for jp in range(16):  # pairs of j blocks
                pSt = [pS.tile([128, 512], F32, tag="psS", name=f"psS_{h2}")
                       for h2 in range(2)]
                pPt = [pP.tile([128, 512], F32R, tag="P32", name=f"P32_{h2}")
                       for h2 in range(2)]
                for h2 in range(2):
                    jb = 2 * jp + h2
                    lo, hi = 64 * h2, 64 * h2 + 64
                    nc.tensor.matmul(
                        pSt[h2][:],
                        lhsT=kt[lo:hi, ts(jb, 128)],
                        rhs=qt[lo:hi, ts(ic, 512)],
                        start=True, stop=True,
                    )
                for h2 in range(2):
                    nc.scalar.activation(
                        pPt[h2][:], pSt[h2][:], mybir.ActivationFunctionType.Exp
                    )
                for h2 in range(2):
                    jb = 2 * jp + h2
                    nc.tensor.matmul(
                        psO[0:65, :],
                        lhsT=VA[:, hh, jb, :],
                        rhs=pPt[h2][:],
                        start=(jb == 0), stop=(jb == 31),
                        skip_group_check=True,
                    )
            o_sb = pOsb.tile([65, 512], F32, tag="o_sb")
            nc.vector.tensor_copy(o_sb[:], psO[0:65, :])
            for t in range(4):
                T = ic * 4 + t
                psT = psmall.tile([128, 65], F32, tag="ps_small")
                nc.tensor.transpose(psT[:], o_sb[:, ts(t, 128)], idf[0:65, 0:65])
                rr = pout.tile([128, 1], F32, tag="rr")
                nc.vector.reciprocal(rr[:], psT[:, 64:65])
                ot = pout.tile([128, 64], F32, tag="ot")
                nc.vector.tensor_scalar_mul(ot[:], psT[:, 0:64], rr[:])
                dst = out.ap()[hh * 64 + 2 * T : hh * 64 + 2 * T + 2, :].rearrange(
                    "c (y x) -> (c y) x", x=64
                )
                nc.sync.dma_start(dst, ot[:])


def _build(loop_reps, static_reps=1, rt=False):
    nc = bacc.Bacc("TRN2", target_bir_lowering=False, debug=False)
    fm = nc.dram_tensor("fm", [256, 4096], F32R, kind="ExternalInput")
    wt = nc.dram_tensor("wt", [256, 384], F32R, kind="ExternalInput")
    pht = nc.dram_tensor("pht", [64, 64], F32R, kind="ExternalInput")
    pwt = nc.dram_tensor("pwt", [64, 64], F32R, kind="ExternalInput")
    out = nc.dram_tensor("out", [128, 4096], F32, kind="ExternalOutput")

    with tile.TileContext(nc) as tc:
        with ExitStack() as ctx:
            singles = ctx.enter_context(tc.tile_pool(name="singles", bufs=1))
            temps = ctx.enter_context(tc.tile_pool(name="temps", bufs=1))
            pP = ctx.enter_context(tc.tile_pool(name="pP", bufs=4))
            pOsb = ctx.enter_context(tc.tile_pool(name="pOsb", bufs=2))
            pout = ctx.enter_context(tc.tile_pool(name="pout", bufs=3))
            psmall = ctx.enter_context(tc.tile_pool(name="psmall", bufs=1 if ATTN_V2 else 2, space="PSUM"))
            ps_bufs, po_bufs = (4, 2) if rt else ((3, 4) if ATTN_V2 else (2, 1))
            pS = ctx.enter_context(tc.tile_pool(name="pS", bufs=ps_bufs, space="PSUM"))
            pO = ctx.enter_context(tc.tile_pool(name="pO", bufs=po_bufs, space="PSUM"))

            idf = singles.tile([128, 128], F32)
            make_identity(nc, idf)
            idr = singles.tile([128, 128], F32R)
            nc.vector.tensor_copy(idr[:], idf[:])

            pools = (singles, temps, pP, pOsb, pout, psmall, pS, pO)
            aps = (fm, wt, pht, pwt, out, idr, idf)
            body = _emit_body_rt if rt else _emit_body
            if loop_reps > 1:
                with tc.For_i(0, loop_reps, 1):
                    body(tc, pools, aps, 0)
            else:
                for rep in range(static_reps):
                    body(tc, pools, aps, rep)
    nc.compile()
    return nc


def build_module(reps=1):
    return _build(1, static_reps=reps)


def build_module_loop(loop_reps):
    return _build(loop_reps)


_module_cache = {}


def get_module(reps=1):
    if reps not in _module_cache:
        _module_cache[reps] = build_module(reps)
    return _module_cache[reps]


def get_module_loop(loop_reps):
    key = ("loop", loop_reps)
    if key not in _module_cache:
        _module_cache[key] = build_module_loop(loop_reps)
    return _module_cache[key]


def make_in_maps(fmap, w_qkv, pos_height, pos_width):
    in_maps = []
    for c in range(N_CORES):
        b, p = c // 2, c % 2
        hA = 2 * p
        segs = []
        for s in range(3):  # q, k, v
            for h in (hA, hA + 1):
                rows = np.asarray(w_qkv[s * 256 + h * 64 : s * 256 + h * 64 + 64, :])
                if s == 0:
                    rows = rows * SCALE
                segs.append(rows)
        wt = np.ascontiguousarray(np.concatenate(segs, 0).T, dtype=np.float32)
        in_maps.append(
            {
                "fm": np.ascontiguousarray(
                    np.asarray(fmap[b]).reshape(256, 4096), dtype=np.float32
                ),
                "wt": wt,
                "pht": np.ascontiguousarray(np.asarray(pos_height).T, dtype=np.float32),
                "pwt": np.ascontiguousarray(np.asarray(pos_width).T, dtype=np.float32),
            }
        )
    return in_maps


def kernel(fmap, w_qkv, pos_height, pos_width):
    nc = get_module(reps=1)
    in_maps = make_in_maps(fmap, w_qkv, pos_height, pos_width)
    res = run_bass_kernel_spmd(nc, in_maps, core_ids=list(range(N_CORES)))
    out = np.empty((4, 256, 64, 64), np.float32)
    for c in range(N_CORES):
        b, p = c // 2, c % 2
        out[b, 128 * p : 128 * p + 128] = res.results[c]["out"].reshape(128, 64, 64)
    return out
